# revision 10
# baseline (speedup 1.0000x reference)
"""Trainium2 Bass kernel for a 12-layer stacked LSTM decoder (teacher forcing).

Problem: B=64, T=128, E=H=512, V=1000, L=12.
Strategy:
  - Host side: BOS-shift + embedding gather, weight re-layout (transposed
    PE tiles, gate-permuted to (i,f,o,g)), bf16 casts.
  - Device side (8 cores, data-parallel over batch, 8 rows/core):
    layer-major chunked wavefront. T is split into C chunks of Tc steps.
    Task (l, c) = run layer l over chunk c: one batched input matmul
    (W_ih stationary, amortized weight load) + Tc sequential recurrent
    steps (W_hh stationary, h^T moving).  Independent tasks are
    interleaved in pairs so each task's elementwise (sigmoid/tanh/DVE)
    latency hides under the partner task's PE work.
  - All matmuls bf16 (fp32 PSUM accumulate); cell state c stays fp32.

Self-contained: only needs numpy/ml_dtypes + the concourse (bass) stack
installed in the container.
"""

import os
import sys

import numpy as np

for _p in ("/opt/trn_rl_repo",):
    if os.path.isdir(_p) and _p not in sys.path:
        sys.path.insert(0, _p)

import ml_dtypes  # noqa: E402

import concourse.bass as bass  # noqa: E402
import concourse.mybir as mybir  # noqa: E402
import concourse.tile as tile  # noqa: E402
from concourse.bass_utils import run_bass_kernel_spmd  # noqa: E402

# ---------------------------------------------------------------- constants
L = 12
B = 64
T = 128
E = 512
H = 512
V = 1000
BOS_ID = 1

NCORES = 8
BL = B // NCORES          # batch rows per core = 8
KT = H // 128             # K tiles per 512-dim contraction = 4
MT = (4 * H) // 128       # M tiles over gate rows = 16
VP = 1024                 # V padded to 8 tiles of 128
VMT = VP // 128           # = 8
TC = 32                   # timesteps per chunk
C = T // TC               # chunks = 4

F32 = mybir.dt.float32
BF16 = mybir.dt.bfloat16
BF16_NP = ml_dtypes.bfloat16

AF = mybir.ActivationFunctionType


# ------------------------------------------------------------- host packing
def _pack_w_stack(w_stack, mt):
    """[L?, rows=mt*128, cols=K*128] -> [L?, 128, KT*mt*128] lhsT tile layout.

    sbuf[p, (k*mt + m)*128 + c] = W[m*128 + c, k*128 + p]
    """
    w = np.asarray(w_stack, np.float32)
    squeeze = w.ndim == 2
    if squeeze:
        w = w[None]
    n = w.shape[0]
    kt = w.shape[2] // 128
    out = (
        w.reshape(n, mt, 128, kt, 128)
        .transpose(0, 4, 3, 1, 2)
        .reshape(n, 128, kt * mt * 128)
        .astype(BF16_NP)
    )
    return out[0] if squeeze else out


_GATE_PERM = np.concatenate(
    [
        np.arange(0, H),          # i
        np.arange(H, 2 * H),      # f
        np.arange(3 * H, 4 * H),  # o
        np.arange(2 * H, 3 * H),  # g
    ]
)


def _prep_shared(inputs):
    """Weight-side tensors, identical on every core."""
    W_ih = np.asarray(inputs["W_ih"], np.float32)[:, _GATE_PERM, :]
    W_hh = np.asarray(inputs["W_hh"], np.float32)[:, _GATE_PERM, :]
    b = (
        np.asarray(inputs["b_ih"], np.float32)
        + np.asarray(inputs["b_hh"], np.float32)
    )[:, _GATE_PERM]

    wih = _pack_w_stack(W_ih, MT)                        # [12,128,8192] bf16
    whh = _pack_w_stack(W_hh, MT)                        # [12,128,8192] bf16
    bias = b.reshape(L, MT, 128).transpose(0, 2, 1).copy()  # [12,128,16] f32

    Wo = np.zeros((VP, H), np.float32)
    Wo[:V] = np.asarray(inputs["W_out"], np.float32)
    wout = _pack_w_stack(Wo, VMT)                        # [128,4096] bf16
    bo = np.zeros((VP,), np.float32)
    bo[:V] = np.asarray(inputs["b_out"], np.float32)
    bout = bo.reshape(VMT, 128).T.copy()                 # [128,8] f32

    return {
        "wih": np.ascontiguousarray(wih),
        "whh": np.ascontiguousarray(whh),
        "bias": np.ascontiguousarray(bias),
        "wout": np.ascontiguousarray(wout),
        "bout": np.ascontiguousarray(bout),
    }


def _prep_core(inputs, ci):
    """Per-core activations: embedded shifted tokens + initial state."""
    tok = np.asarray(inputs["target_token_ids"])
    tokens = np.concatenate(
        [np.full((B, 1), BOS_ID, tok.dtype), tok[:, :-1]], axis=1
    )
    emb = np.asarray(inputs["embed_table"], np.float32)
    bs = slice(ci * BL, (ci + 1) * BL)

    e = emb[tokens[bs]]                                  # [8, T, 512] f32
    x0 = (
        e.reshape(BL, T, KT, 128)
        .transpose(3, 1, 2, 0)                           # [p, t, k, b]
        .reshape(128, T * KT * BL)
        .astype(BF16_NP)
    )

    h0 = np.asarray(inputs["h0"], np.float32)[bs]        # [8, 512]
    c0 = np.asarray(inputs["c0"], np.float32)[bs]
    h0T = h0.reshape(BL, KT, 128).transpose(2, 1, 0).reshape(128, KT * BL)
    c0T = c0.reshape(BL, KT, 128).transpose(2, 1, 0).reshape(128, KT * BL)

    return {
        "x0": np.ascontiguousarray(x0),
        "h0T": np.ascontiguousarray(h0T.astype(BF16_NP)),
        "c0T": np.ascontiguousarray(c0T.astype(np.float32)),
    }


def _unpack_logits(lt):
    """[VMT, 128, T*BL] f32 -> [BL, T, V] f32."""
    lt = np.asarray(lt, np.float32).reshape(VMT, 128, T, BL)
    return lt.transpose(3, 2, 0, 1).reshape(BL, T, VP)[:, :, :V]


# ----------------------------------------------------------- task schedule
def _greedy_pairs():
    """Topological greedy pairing of tasks (l, c)."""
    done = set()
    todo = {(l, c) for l in range(L) for c in range(C)}
    groups = []

    def ready(t):
        l, c = t
        return (l == 0 or (l - 1, c) in done) and (c == 0 or (l, c - 1) in done)

    while todo:
        r = sorted([t for t in todo if ready(t)], key=lambda t: (t[0] + t[1], t[0]))
        grp = tuple(r[:2]) if len(r) >= 2 else (r[0],)
        groups.append(grp)
        for t in grp:
            done.add(t)
            todo.discard(t)
    return groups


# ------------------------------------------------------------ device kernel
_WAIT_LIMITS = {}


def _split_excess_waits(nc, default_limit=1):
    """Walrus codegen allows only a small number of sync waits per TPB
    instruction (1 for Activation, 2 for most others).  Move any excess
    onto standalone EventSemaphore instructions inserted just before the
    over-subscribed instruction (same engine stream, so semantics are
    identical)."""
    import bass_rust as _br

    n_split = 0
    for fn in nc.m.functions:
        for bb in fn.blocks:
            insts = list(bb.instructions)
            out = []
            changed = False
            for inst in insts:
                tname = type(inst).__name__
                limit = _WAIT_LIMITS.get(tname, default_limit)
                si = getattr(inst, "sync_info", None)
                ow = list(si.on_wait) if si is not None and si.on_wait else []
                if limit is not None and len(ow) > limit:
                    keep = ow[-limit:] if limit else []
                    rest = ow[:len(ow) - limit]
                    for j, w in enumerate(rest):
                        ev = mybir.InstEventSemaphore(
                            name=f"{inst.name}_wsplit{j}"
                        )
                        ev.engine = inst.engine
                        ev.sync_info = _br.SyncInfo(on_wait=[w], on_update=[])
                        out.append(ev)
                        n_split += 1
                    inst.sync_info = _br.SyncInfo(
                        on_wait=keep, on_update=list(si.on_update or [])
                    )
                    changed = True
                out.append(inst)
            if changed:
                bb.instructions = out
    return n_split


def _build_program():
    nc = bass.Bass(
        "TRN2", target_bir_lowering=False, debug=False, enable_asserts=False
    )

    wih_d = nc.dram_tensor("wih", [L, 128, KT * MT * 128], BF16,
                           kind="ExternalInput").ap()
    whh_d = nc.dram_tensor("whh", [L, 128, KT * MT * 128], BF16,
                           kind="ExternalInput").ap()
    bias_d = nc.dram_tensor("bias", [L, 128, MT], F32,
                            kind="ExternalInput").ap()
    x0_d = nc.dram_tensor("x0", [128, T * KT * BL], BF16,
                          kind="ExternalInput").ap()
    h0_d = nc.dram_tensor("h0T", [128, KT * BL], BF16,
                          kind="ExternalInput").ap()
    c0_d = nc.dram_tensor("c0T", [128, KT * BL], F32,
                          kind="ExternalInput").ap()
    wout_d = nc.dram_tensor("wout", [128, KT * VMT * 128], BF16,
                            kind="ExternalInput").ap()
    bout_d = nc.dram_tensor("bout", [128, VMT], F32,
                            kind="ExternalInput").ap()
    lg_d = nc.dram_tensor("logitsT", [VMT, 128, T * BL], F32,
                          kind="ExternalOutput").ap()

    XW = TC * KT * BL     # x-chunk free width  = 1024
    SW = KT * BL          # per-step h width    = 32
    AW = TC * MT * BL     # A-chunk free width  = 4096

    with tile.TileContext(nc) as tc:
        with (
            tc.tile_pool(name="wpool", bufs=6) as wpool,
            tc.tile_pool(name="xpool", bufs=10) as xpool,
            tc.tile_pool(name="apool", bufs=3) as apool,
            tc.tile_pool(name="bpool", bufs=4) as bpool,
            tc.tile_pool(name="const", bufs=1) as cpool,
            tc.tile_pool(name="gpool", bufs=4) as gpool,
            tc.tile_pool(name="spool", bufs=4) as spool,
            tc.tile_pool(name="gps", bufs=4, space="PSUM") as gps,
            tc.tile_pool(name="aps", bufs=2, space="PSUM") as aps,
        ):
            # persistent: initial state + output weights
            h0_t = cpool.tile([128, SW], BF16, tag="h0")
            nc.sync.dma_start(h0_t[:], h0_d)
            c0_t = cpool.tile([128, SW], F32, tag="c0")
            nc.sync.dma_start(c0_t[:], c0_d)
            wout_t = cpool.tile([128, KT * VMT * 128], BF16, tag="wout")
            nc.sync.dma_start(wout_t[:], wout_d)
            bout_t = cpool.tile([128, VMT], F32, tag="bout")
            nc.sync.dma_start(bout_t[:], bout_d)
            bias_all = cpool.tile([128, L * MT], F32, tag="bias_all")
            nc.sync.dma_start(
                bias_all[:], bias_d.rearrange("l p m -> p l m")
            )

            cstate = {}
            for l in range(L):
                cstate[l] = cpool.tile(
                    [128, SW], F32, tag=f"cst{l}", name=f"cst{l}"
                )

            chunk_out = {}

            def emit_prologue(task):
                l, c = task
                st = {}
                wih_t = wpool.tile([128, KT * MT * 128], BF16, tag="w")
                nc.sync.dma_start(wih_t[:], wih_d[l])
                whh_t = wpool.tile([128, KT * MT * 128], BF16, tag="w")
                nc.sync.dma_start(whh_t[:], whh_d[l])
                bias_t = bias_all[:, l * MT:(l + 1) * MT]

                if l == 0:
                    xin = xpool.tile([128, XW], BF16, tag="xchunk")
                    nc.sync.dma_start(xin[:], x0_d[:, c * XW:(c + 1) * XW])
                else:
                    xin = chunk_out[(l - 1, c)]
                xout = xpool.tile([128, XW], BF16, tag="xchunk")
                chunk_out[(l, c)] = xout

                # batched input matmul: A[t,m,b] = (x W_ih^T + b)
                a_t = apool.tile([128, AW], F32, tag="A")
                xin_k = xin[:].rearrange("p (t k b) -> p k t b", t=TC, k=KT)
                a_mt = a_t[:].rearrange("p (t m b) -> p m t b", t=TC, m=MT)
                for m in range(MT):
                    ps = aps.tile([128, TC * BL], F32, tag="aps")
                    for k in range(KT):
                        nc.tensor.matmul(
                            ps[:],
                            lhsT=wih_t[:, (k * MT + m) * 128:(k * MT + m + 1) * 128],
                            rhs=xin_k[:, k],
                            start=(k == 0),
                            stop=(k == KT - 1),
                        )
                    nc.scalar.activation(
                        a_mt[:, m],
                        ps[:].rearrange("p (t b) -> p t b", t=TC),
                        AF.Identity,
                        bias=bias_t[:, m:m + 1],
                    )

                st.update(whh=whh_t, xin=xin, xout=xout, A=a_t, l=l, c=c)
                return st

            def emit_step(st, t):
                l = st["l"]
                whh_t = st["whh"]
                xout = st["xout"]

                if t == 0:
                    if st["c"] == 0:
                        rhs = h0_t[:]
                    else:
                        prev = chunk_out[(l, st["c"] - 1)]
                        rhs = prev[:, (TC - 1) * SW:TC * SW]
                else:
                    rhs = xout[:, (t - 1) * SW:t * SW]

                g_ps = gps.tile([128, MT * BL], F32, tag="gps")
                for m in range(MT):
                    for k in range(KT):
                        nc.tensor.matmul(
                            g_ps[:, m * BL:(m + 1) * BL],
                            lhsT=whh_t[:, (k * MT + m) * 128:(k * MT + m + 1) * 128],
                            rhs=rhs[:, k * BL:(k + 1) * BL],
                            start=(k == 0),
                            stop=(k == KT - 1),
                        )

                GW = MT * BL          # 128
                QW = GW // 4          # 32 per gate
                g = gpool.tile([128, GW], F32, tag="g")
                nc.vector.tensor_add(
                    g[:], g_ps[:], st["A"][:, t * GW:(t + 1) * GW]
                )
                sig = spool.tile([128, 3 * QW], F32, tag="sig")
                nc.scalar.activation(sig[:], g[:, 0:3 * QW], AF.Sigmoid)
                tg = spool.tile([128, QW], F32, tag="tg")
                nc.scalar.activation(tg[:], g[:, 3 * QW:GW], AF.Tanh)

                t1 = spool.tile([128, QW], F32, tag="t1")
                nc.vector.tensor_mul(t1[:], sig[:, 0:QW], tg[:])
                t2 = spool.tile([128, QW], F32, tag="t2")
                nc.vector.tensor_mul(t2[:], sig[:, QW:2 * QW], cstate[l][:])
                nc.vector.tensor_add(cstate[l][:], t1[:], t2[:])
                tcn = spool.tile([128, QW], F32, tag="tcn")
                nc.scalar.activation(tcn[:], cstate[l][:], AF.Tanh)
                nc.vector.tensor_mul(
                    xout[:, t * SW:(t + 1) * SW], sig[:, 2 * QW:3 * QW], tcn[:]
                )

            def emit_epilogue(st):
                # output projection for the last layer's chunk
                l, c = st["l"], st["c"]
                if l != L - 1:
                    return
                xo_k = st["xout"][:].rearrange(
                    "p (t k b) -> p k t b", t=TC, k=KT
                )
                for m in range(VMT):
                    ps = aps.tile([128, TC * BL], F32, tag="aps")
                    for k in range(KT):
                        nc.tensor.matmul(
                            ps[:],
                            lhsT=wout_t[:, (k * VMT + m) * 128:(k * VMT + m + 1) * 128],
                            rhs=xo_k[:, k],
                            start=(k == 0),
                            stop=(k == KT - 1),
                        )
                    lg_s = gpool.tile([128, TC * BL], F32, tag="lgout")
                    nc.scalar.activation(
                        lg_s[:], ps[:], AF.Identity, bias=bout_t[:, m:m + 1]
                    )
                    nc.sync.dma_start(
                        lg_d[m][:, c * TC * BL:(c + 1) * TC * BL], lg_s[:]
                    )

            # c-state init (once, before any task)
            for l in range(L):
                nc.vector.tensor_copy(cstate[l][:], c0_t[:])

            for grp in _greedy_pairs():
                sts = [emit_prologue(task) for task in grp]
                for t in range(TC):
                    for st in sts:
                        emit_step(st, t)
                for st in sts:
                    emit_epilogue(st)

    _split_excess_waits(nc)
    return nc


# ---------------------------------------------------------------- interface
_CACHE = {}


def _get_program():
    if "nc" not in _CACHE:
        _CACHE["nc"] = _build_program()
    return _CACHE["nc"]


def run(trace=False, **inputs):
    nc = _get_program()
    shared = _prep_shared(inputs)
    in_maps = []
    for ci in range(NCORES):
        m = dict(shared)
        m.update(_prep_core(inputs, ci))
        in_maps.append(m)

    res = run_bass_kernel_spmd(nc, in_maps, list(range(NCORES)), trace=trace)
    out = np.empty((B, T, V), np.float32)
    for ci in range(NCORES):
        out[ci * BL:(ci + 1) * BL] = _unpack_logits(res.results[ci]["logitsT"])
    return out, res


def kernel(**inputs):
    return run(trace=False, **inputs)[0]


# revision 11
# speedup vs baseline: 1.0482x; 1.0482x over previous
"""Trainium2 Bass kernel for a 12-layer stacked LSTM decoder (teacher forcing).

Problem: B=64, T=128, E=H=512, V=1000, L=12.
Strategy:
  - Host side: BOS-shift + embedding gather, weight re-layout (transposed
    PE tiles, gate-permuted to (i,f,o,g)), bf16 casts.
  - Device side (8 cores, data-parallel over batch, 8 rows/core):
    layer-major chunked wavefront. T is split into C chunks of Tc steps.
    Task (l, c) = run layer l over chunk c: one batched input matmul
    (W_ih stationary, amortized weight load) + Tc sequential recurrent
    steps (W_hh stationary, h^T moving).  Independent tasks are
    interleaved in pairs so each task's elementwise (sigmoid/tanh/DVE)
    latency hides under the partner task's PE work.
  - All matmuls bf16 (fp32 PSUM accumulate); cell state c stays fp32.

Self-contained: only needs numpy/ml_dtypes + the concourse (bass) stack
installed in the container.
"""

import os
import sys

import numpy as np

for _p in ("/opt/trn_rl_repo",):
    if os.path.isdir(_p) and _p not in sys.path:
        sys.path.insert(0, _p)

import ml_dtypes  # noqa: E402

import concourse.bass as bass  # noqa: E402
import concourse.mybir as mybir  # noqa: E402
import concourse.tile as tile  # noqa: E402
from concourse.bass_utils import run_bass_kernel_spmd  # noqa: E402

# ---------------------------------------------------------------- constants
L = 12
B = 64
T = 128
E = 512
H = 512
V = 1000
BOS_ID = 1

NCORES = 8
BL = B // NCORES          # batch rows per core = 8
KT = H // 128             # K tiles per 512-dim contraction = 4
MT = (4 * H) // 128       # M tiles over gate rows = 16
VP = 1024                 # V padded to 8 tiles of 128
VMT = VP // 128           # = 8
TC = 32                   # timesteps per chunk
C = T // TC               # chunks = 4

F32 = mybir.dt.float32
BF16 = mybir.dt.bfloat16
BF16_NP = ml_dtypes.bfloat16

AF = mybir.ActivationFunctionType


# ------------------------------------------------------------- host packing
def _pack_w_stack(w_stack, mt):
    """[L?, rows=mt*128, cols=K*128] -> [L?, 128, KT*mt*128] lhsT tile layout.

    sbuf[p, (k*mt + m)*128 + c] = W[m*128 + c, k*128 + p]
    """
    w = np.asarray(w_stack, np.float32)
    squeeze = w.ndim == 2
    if squeeze:
        w = w[None]
    n = w.shape[0]
    kt = w.shape[2] // 128
    out = (
        w.reshape(n, mt, 128, kt, 128)
        .transpose(0, 4, 3, 1, 2)
        .reshape(n, 128, kt * mt * 128)
        .astype(BF16_NP)
    )
    return out[0] if squeeze else out


_GATE_PERM = np.concatenate(
    [
        np.arange(0, H),          # i
        np.arange(H, 2 * H),      # f
        np.arange(3 * H, 4 * H),  # o
        np.arange(2 * H, 3 * H),  # g
    ]
)


def _prep_shared(inputs):
    """Weight-side tensors, identical on every core."""
    W_ih = np.asarray(inputs["W_ih"], np.float32)[:, _GATE_PERM, :]
    W_hh = np.asarray(inputs["W_hh"], np.float32)[:, _GATE_PERM, :]
    b = (
        np.asarray(inputs["b_ih"], np.float32)
        + np.asarray(inputs["b_hh"], np.float32)
    )[:, _GATE_PERM]

    wih = _pack_w_stack(W_ih, MT)                        # [12,128,8192] bf16
    whh = _pack_w_stack(W_hh, MT)                        # [12,128,8192] bf16
    bias = b.reshape(L, MT, 128).transpose(0, 2, 1).copy()  # [12,128,16] f32

    Wo = np.zeros((VP, H), np.float32)
    Wo[:V] = np.asarray(inputs["W_out"], np.float32)
    wout = _pack_w_stack(Wo, VMT)                        # [128,4096] bf16
    bo = np.zeros((VP,), np.float32)
    bo[:V] = np.asarray(inputs["b_out"], np.float32)
    bout = bo.reshape(VMT, 128).T.copy()                 # [128,8] f32

    return {
        "wih": np.ascontiguousarray(wih),
        "whh": np.ascontiguousarray(whh),
        "bias": np.ascontiguousarray(bias),
        "wout": np.ascontiguousarray(wout),
        "bout": np.ascontiguousarray(bout),
    }


def _prep_core(inputs, ci):
    """Per-core activations: embedded shifted tokens + initial state."""
    tok = np.asarray(inputs["target_token_ids"])
    tokens = np.concatenate(
        [np.full((B, 1), BOS_ID, tok.dtype), tok[:, :-1]], axis=1
    )
    emb = np.asarray(inputs["embed_table"], np.float32)
    bs = slice(ci * BL, (ci + 1) * BL)

    e = emb[tokens[bs]]                                  # [8, T, 512] f32
    x0 = (
        e.reshape(BL, T, KT, 128)
        .transpose(3, 1, 2, 0)                           # [p, t, k, b]
        .reshape(128, T * KT * BL)
        .astype(BF16_NP)
    )

    h0 = np.asarray(inputs["h0"], np.float32)[bs]        # [8, 512]
    c0 = np.asarray(inputs["c0"], np.float32)[bs]
    h0T = h0.reshape(BL, KT, 128).transpose(2, 1, 0).reshape(128, KT * BL)
    c0T = c0.reshape(BL, KT, 128).transpose(2, 1, 0).reshape(128, KT * BL)

    return {
        "x0": np.ascontiguousarray(x0),
        "h0T": np.ascontiguousarray(h0T.astype(BF16_NP)),
        "c0T": np.ascontiguousarray(c0T.astype(np.float32)),
    }


def _unpack_logits(lt):
    """[VMT, 128, T*BL] f32 -> [BL, T, V] f32."""
    lt = np.asarray(lt, np.float32).reshape(VMT, 128, T, BL)
    return lt.transpose(3, 2, 0, 1).reshape(BL, T, VP)[:, :, :V]


# ----------------------------------------------------------- task schedule
def _greedy_groups(width=3):
    """Topological greedy grouping of independent tasks (l, c)."""
    done = set()
    todo = {(l, c) for l in range(L) for c in range(C)}
    groups = []

    def ready(t):
        l, c = t
        return (l == 0 or (l - 1, c) in done) and (c == 0 or (l, c - 1) in done)

    while todo:
        r = sorted([t for t in todo if ready(t)], key=lambda t: (t[0] + t[1], t[0]))
        grp = tuple(r[:width]) if r else ()
        groups.append(grp)
        for t in grp:
            done.add(t)
            todo.discard(t)
    return groups


# ------------------------------------------------------------ device kernel
_WAIT_LIMITS = {}


def _split_excess_waits(nc, default_limit=1):
    """Walrus codegen allows only a small number of sync waits per TPB
    instruction (1 for Activation, 2 for most others).  Move any excess
    onto standalone EventSemaphore instructions inserted just before the
    over-subscribed instruction (same engine stream, so semantics are
    identical)."""
    import bass_rust as _br

    n_split = 0
    for fn in nc.m.functions:
        for bb in fn.blocks:
            insts = list(bb.instructions)
            out = []
            changed = False
            for inst in insts:
                tname = type(inst).__name__
                limit = _WAIT_LIMITS.get(tname, default_limit)
                si = getattr(inst, "sync_info", None)
                ow = list(si.on_wait) if si is not None and si.on_wait else []
                if limit is not None and len(ow) > limit:
                    keep = ow[-limit:] if limit else []
                    rest = ow[:len(ow) - limit]
                    for j, w in enumerate(rest):
                        ev = mybir.InstEventSemaphore(
                            name=f"{inst.name}_wsplit{j}"
                        )
                        ev.engine = inst.engine
                        ev.sync_info = _br.SyncInfo(on_wait=[w], on_update=[])
                        out.append(ev)
                        n_split += 1
                    inst.sync_info = _br.SyncInfo(
                        on_wait=keep, on_update=list(si.on_update or [])
                    )
                    changed = True
                out.append(inst)
            if changed:
                bb.instructions = out
    return n_split


def _build_program():
    nc = bass.Bass(
        "TRN2", target_bir_lowering=False, debug=False, enable_asserts=False
    )

    wih_d = nc.dram_tensor("wih", [L, 128, KT * MT * 128], BF16,
                           kind="ExternalInput").ap()
    whh_d = nc.dram_tensor("whh", [L, 128, KT * MT * 128], BF16,
                           kind="ExternalInput").ap()
    bias_d = nc.dram_tensor("bias", [L, 128, MT], F32,
                            kind="ExternalInput").ap()
    x0_d = nc.dram_tensor("x0", [128, T * KT * BL], BF16,
                          kind="ExternalInput").ap()
    h0_d = nc.dram_tensor("h0T", [128, KT * BL], BF16,
                          kind="ExternalInput").ap()
    c0_d = nc.dram_tensor("c0T", [128, KT * BL], F32,
                          kind="ExternalInput").ap()
    wout_d = nc.dram_tensor("wout", [128, KT * VMT * 128], BF16,
                            kind="ExternalInput").ap()
    bout_d = nc.dram_tensor("bout", [128, VMT], F32,
                            kind="ExternalInput").ap()
    lg_d = nc.dram_tensor("logitsT", [VMT, 128, T * BL], F32,
                          kind="ExternalOutput").ap()

    XW = TC * KT * BL     # x-chunk free width  = 1024
    SW = KT * BL          # per-step h width    = 32
    AW = TC * MT * BL     # A-chunk free width  = 4096

    with tile.TileContext(nc) as tc:
        with (
            tc.tile_pool(name="wpool", bufs=8) as wpool,
            tc.tile_pool(name="xpool", bufs=12) as xpool,
            tc.tile_pool(name="apool", bufs=4) as apool,
            tc.tile_pool(name="bpool", bufs=4) as bpool,
            tc.tile_pool(name="const", bufs=1) as cpool,
            tc.tile_pool(name="gpool", bufs=4) as gpool,
            tc.tile_pool(name="spool", bufs=4) as spool,
            tc.tile_pool(name="gps", bufs=6, space="PSUM") as gps,
            tc.tile_pool(name="aps", bufs=2, space="PSUM") as aps,
        ):
            # persistent: initial state + output weights
            h0_t = cpool.tile([128, SW], BF16, tag="h0")
            nc.sync.dma_start(h0_t[:], h0_d)
            c0_t = cpool.tile([128, SW], F32, tag="c0")
            nc.sync.dma_start(c0_t[:], c0_d)
            wout_t = cpool.tile([128, KT * VMT * 128], BF16, tag="wout")
            nc.sync.dma_start(wout_t[:], wout_d)
            bout_t = cpool.tile([128, VMT], F32, tag="bout")
            nc.sync.dma_start(bout_t[:], bout_d)
            bias_all = cpool.tile([128, L * MT], F32, tag="bias_all")
            nc.sync.dma_start(
                bias_all[:], bias_d.rearrange("l p m -> p l m")
            )

            cstate = {}
            for l in range(L):
                cstate[l] = cpool.tile(
                    [128, SW], F32, tag=f"cst{l}", name=f"cst{l}"
                )

            chunk_out = {}

            def emit_prologue(task):
                l, c = task
                st = {}
                wih_t = wpool.tile([128, KT * MT * 128], BF16, tag="w")
                nc.sync.dma_start(wih_t[:], wih_d[l])
                whh_t = wpool.tile([128, KT * MT * 128], BF16, tag="w")
                nc.sync.dma_start(whh_t[:], whh_d[l])
                bias_t = bias_all[:, l * MT:(l + 1) * MT]

                if l == 0:
                    xin = xpool.tile([128, XW], BF16, tag="xchunk")
                    nc.sync.dma_start(xin[:], x0_d[:, c * XW:(c + 1) * XW])
                else:
                    xin = chunk_out[(l - 1, c)]
                xout = xpool.tile([128, XW], BF16, tag="xchunk")
                chunk_out[(l, c)] = xout

                # batched input matmul: A[t,m,b] = (x W_ih^T + b)
                a_t = apool.tile([128, AW], BF16, tag="A")
                xin_k = xin[:].rearrange("p (t k b) -> p k t b", t=TC, k=KT)
                a_mt = a_t[:].rearrange("p (t m b) -> p m t b", t=TC, m=MT)
                for m in range(MT):
                    ps = aps.tile([128, TC * BL], F32, tag="aps")
                    for k in range(KT):
                        nc.tensor.matmul(
                            ps[:],
                            lhsT=wih_t[:, (k * MT + m) * 128:(k * MT + m + 1) * 128],
                            rhs=xin_k[:, k],
                            start=(k == 0),
                            stop=(k == KT - 1),
                        )
                    nc.scalar.activation(
                        a_mt[:, m],
                        ps[:].rearrange("p (t b) -> p t b", t=TC),
                        AF.Identity,
                        bias=bias_t[:, m:m + 1],
                    )

                st.update(whh=whh_t, xin=xin, xout=xout, A=a_t, l=l, c=c)
                return st

            def emit_step(st, t):
                l = st["l"]
                whh_t = st["whh"]
                xout = st["xout"]

                if t == 0:
                    if st["c"] == 0:
                        rhs = h0_t[:]
                    else:
                        prev = chunk_out[(l, st["c"] - 1)]
                        rhs = prev[:, (TC - 1) * SW:TC * SW]
                else:
                    rhs = xout[:, (t - 1) * SW:t * SW]

                g_ps = gps.tile([128, MT * BL], F32, tag="gps")
                for m in range(MT):
                    for k in range(KT):
                        nc.tensor.matmul(
                            g_ps[:, m * BL:(m + 1) * BL],
                            lhsT=whh_t[:, (k * MT + m) * 128:(k * MT + m + 1) * 128],
                            rhs=rhs[:, k * BL:(k + 1) * BL],
                            start=(k == 0),
                            stop=(k == KT - 1),
                        )

                GW = MT * BL          # 128
                QW = GW // 4          # 32 per gate
                g = gpool.tile([128, GW], F32, tag="g")
                nc.vector.tensor_add(
                    g[:], g_ps[:], st["A"][:, t * GW:(t + 1) * GW]
                )
                sig = spool.tile([128, 3 * QW], F32, tag="sig")
                nc.scalar.activation(sig[:], g[:, 0:3 * QW], AF.Sigmoid)
                tg = spool.tile([128, QW], F32, tag="tg")
                nc.scalar.activation(tg[:], g[:, 3 * QW:GW], AF.Tanh)

                t1 = spool.tile([128, QW], F32, tag="t1")
                nc.vector.tensor_mul(t1[:], sig[:, 0:QW], tg[:])
                t2 = spool.tile([128, QW], F32, tag="t2")
                nc.vector.tensor_mul(t2[:], sig[:, QW:2 * QW], cstate[l][:])
                nc.vector.tensor_add(cstate[l][:], t1[:], t2[:])
                tcn = spool.tile([128, QW], F32, tag="tcn")
                nc.scalar.activation(tcn[:], cstate[l][:], AF.Tanh)
                nc.vector.tensor_mul(
                    xout[:, t * SW:(t + 1) * SW], sig[:, 2 * QW:3 * QW], tcn[:]
                )

            def emit_epilogue(st):
                # output projection for the last layer's chunk
                l, c = st["l"], st["c"]
                if l != L - 1:
                    return
                xo_k = st["xout"][:].rearrange(
                    "p (t k b) -> p k t b", t=TC, k=KT
                )
                for m in range(VMT):
                    ps = aps.tile([128, TC * BL], F32, tag="aps")
                    for k in range(KT):
                        nc.tensor.matmul(
                            ps[:],
                            lhsT=wout_t[:, (k * VMT + m) * 128:(k * VMT + m + 1) * 128],
                            rhs=xo_k[:, k],
                            start=(k == 0),
                            stop=(k == KT - 1),
                        )
                    lg_s = gpool.tile([128, TC * BL], F32, tag="lgout")
                    nc.scalar.activation(
                        lg_s[:], ps[:], AF.Identity, bias=bout_t[:, m:m + 1]
                    )
                    nc.sync.dma_start(
                        lg_d[m][:, c * TC * BL:(c + 1) * TC * BL], lg_s[:]
                    )

            # c-state init (once, before any task)
            for l in range(L):
                nc.vector.tensor_copy(cstate[l][:], c0_t[:])

            for grp in _greedy_groups():
                sts = [emit_prologue(task) for task in grp]
                for t in range(TC):
                    for st in sts:
                        emit_step(st, t)
                for st in sts:
                    emit_epilogue(st)

    _split_excess_waits(nc)
    return nc


# ---------------------------------------------------------------- interface
_CACHE = {}


def _get_program():
    if "nc" not in _CACHE:
        _CACHE["nc"] = _build_program()
    return _CACHE["nc"]


def run(trace=False, **inputs):
    nc = _get_program()
    shared = _prep_shared(inputs)
    in_maps = []
    for ci in range(NCORES):
        m = dict(shared)
        m.update(_prep_core(inputs, ci))
        in_maps.append(m)

    res = run_bass_kernel_spmd(nc, in_maps, list(range(NCORES)), trace=trace)
    out = np.empty((B, T, V), np.float32)
    for ci in range(NCORES):
        out[ci * BL:(ci + 1) * BL] = _unpack_logits(res.results[ci]["logitsT"])
    return out, res


def kernel(**inputs):
    return run(trace=False, **inputs)[0]


# revision 16
# speedup vs baseline: 1.2404x; 1.1834x over previous
"""Trainium2 Bass kernel for a 12-layer stacked LSTM decoder (teacher forcing).

Problem: B=64, T=128, E=H=512, V=1000, L=12.
Strategy:
  - Host side: BOS-shift + embedding gather, weight re-layout (transposed
    PE tiles, gate-permuted to (i,f,o,g)), bf16 casts.
  - Device side (8 cores, data-parallel over batch, 8 rows/core):
    layer-major chunked wavefront. T is split into C chunks of Tc steps.
    Task (l, c) = run layer l over chunk c: one batched input matmul
    (W_ih stationary, amortized weight load) + Tc sequential recurrent
    steps (W_hh stationary, h^T moving).  Independent tasks are
    interleaved in pairs so each task's elementwise (sigmoid/tanh/DVE)
    latency hides under the partner task's PE work.
  - All matmuls bf16 (fp32 PSUM accumulate); cell state c stays fp32.

Self-contained: only needs numpy/ml_dtypes + the concourse (bass) stack
installed in the container.
"""

import os
import sys

import numpy as np

for _p in ("/opt/trn_rl_repo",):
    if os.path.isdir(_p) and _p not in sys.path:
        sys.path.insert(0, _p)

import ml_dtypes  # noqa: E402

import concourse.bass as bass  # noqa: E402
import concourse.mybir as mybir  # noqa: E402
import concourse.tile as tile  # noqa: E402
from concourse.bass_utils import run_bass_kernel_spmd  # noqa: E402

# ---------------------------------------------------------------- constants
L = 12
B = 64
T = 128
E = 512
H = 512
V = 1000
BOS_ID = 1

NCORES = 8
BL = B // NCORES          # batch rows per core = 8
KT = H // 128             # K tiles per 512-dim contraction = 4
MT = (4 * H) // 128       # M tiles over gate rows = 16
VP = 1024                 # V padded to 8 tiles of 128
VMT = VP // 128           # = 8
TC = 32                   # timesteps per chunk
C = T // TC               # chunks = 4

F32 = mybir.dt.float32
BF16 = mybir.dt.bfloat16
BF16_NP = ml_dtypes.bfloat16

AF = mybir.ActivationFunctionType


# ------------------------------------------------------------- host packing
def _pack_w_stack(w_stack, mt):
    """[L?, rows=mt*128, cols=K*128] -> [L?, 128, KT*mt*128] lhsT tile layout.

    sbuf[p, (k*mt + m)*128 + c] = W[m*128 + c, k*128 + p]
    """
    w = np.asarray(w_stack, np.float32)
    squeeze = w.ndim == 2
    if squeeze:
        w = w[None]
    n = w.shape[0]
    kt = w.shape[2] // 128
    out = (
        w.reshape(n, mt, 128, kt, 128)
        .transpose(0, 4, 3, 1, 2)
        .reshape(n, 128, kt * mt * 128)
        .astype(BF16_NP)
    )
    return out[0] if squeeze else out


_GATE_PERM = np.concatenate(
    [
        np.arange(0, H),          # i
        np.arange(H, 2 * H),      # f
        np.arange(3 * H, 4 * H),  # o
        np.arange(2 * H, 3 * H),  # g
    ]
)


def _prep_shared(inputs):
    """Weight-side tensors, identical on every core."""
    W_ih = np.asarray(inputs["W_ih"], np.float32)[:, _GATE_PERM, :]
    W_hh = np.asarray(inputs["W_hh"], np.float32)[:, _GATE_PERM, :]
    b = (
        np.asarray(inputs["b_ih"], np.float32)
        + np.asarray(inputs["b_hh"], np.float32)
    )[:, _GATE_PERM]

    wih = _pack_w_stack(W_ih, MT)                        # [12,128,8192] bf16
    whh = _pack_w_stack(W_hh, MT)                        # [12,128,8192] bf16
    bias = b.reshape(L, MT, 128).transpose(0, 2, 1).copy()  # [12,128,16] f32

    Wo = np.zeros((VP, H), np.float32)
    Wo[:V] = np.asarray(inputs["W_out"], np.float32)
    wout = _pack_w_stack(Wo, VMT)                        # [128,4096] bf16
    bo = np.zeros((VP,), np.float32)
    bo[:V] = np.asarray(inputs["b_out"], np.float32)
    bout = bo.reshape(VMT, 128).T.copy()                 # [128,8] f32

    return {
        "ident": np.ascontiguousarray(np.eye(128, dtype=BF16_NP)),
        "wih": np.ascontiguousarray(wih),
        "whh": np.ascontiguousarray(whh),
        "bias": np.ascontiguousarray(bias),
        "wout": np.ascontiguousarray(wout),
        "bout": np.ascontiguousarray(bout),
    }


def _prep_core(inputs, ci):
    """Per-core activations: embedded shifted tokens + initial state."""
    tok = np.asarray(inputs["target_token_ids"])
    tokens = np.concatenate(
        [np.full((B, 1), BOS_ID, tok.dtype), tok[:, :-1]], axis=1
    )
    emb = np.asarray(inputs["embed_table"], np.float32)
    bs = slice(ci * BL, (ci + 1) * BL)

    e = emb[tokens[bs]]                                  # [8, T, 512] f32
    x0 = (
        e.reshape(BL, T, KT, 128)
        .transpose(3, 1, 2, 0)                           # [p, t, k, b]
        .reshape(128, T * KT * BL)
        .astype(BF16_NP)
    )

    h0 = np.asarray(inputs["h0"], np.float32)[bs]        # [8, 512]
    c0 = np.asarray(inputs["c0"], np.float32)[bs]
    h0T = h0.reshape(BL, KT, 128).transpose(2, 1, 0).reshape(128, KT * BL)
    c0T = c0.reshape(BL, KT, 128).transpose(2, 1, 0).reshape(128, KT * BL)

    return {
        "x0": np.ascontiguousarray(x0),
        "h0T": np.ascontiguousarray(h0T.astype(BF16_NP)),
        "c0T": np.ascontiguousarray(c0T.astype(np.float32)),
    }


def _unpack_logits(lt):
    """[VMT, 128, T*BL] f32 -> [BL, T, V] f32."""
    lt = np.asarray(lt, np.float32).reshape(VMT, 128, T, BL)
    return lt.transpose(3, 2, 0, 1).reshape(BL, T, VP)[:, :, :V]


# ----------------------------------------------------------- task schedule
def _greedy_groups(width=3):
    """Topological greedy grouping of independent tasks (l, c)."""
    done = set()
    todo = {(l, c) for l in range(L) for c in range(C)}
    groups = []

    def ready(t):
        l, c = t
        return (l == 0 or (l - 1, c) in done) and (c == 0 or (l, c - 1) in done)

    while todo:
        r = sorted([t for t in todo if ready(t)], key=lambda t: (t[0] + t[1], t[0]))
        grp = tuple(r[:width]) if r else ()
        groups.append(grp)
        for t in grp:
            done.add(t)
            todo.discard(t)
    return groups


# ------------------------------------------------------------ device kernel
_WAIT_LIMITS = {}


def _split_excess_waits(nc, default_limit=1):
    """Walrus codegen allows only a small number of sync waits per TPB
    instruction (1 for Activation, 2 for most others).  Move any excess
    onto standalone EventSemaphore instructions inserted just before the
    over-subscribed instruction (same engine stream, so semantics are
    identical)."""
    import bass_rust as _br

    n_split = 0
    for fn in nc.m.functions:
        for bb in fn.blocks:
            insts = list(bb.instructions)
            out = []
            changed = False
            for inst in insts:
                tname = type(inst).__name__
                limit = _WAIT_LIMITS.get(tname, default_limit)
                si = getattr(inst, "sync_info", None)
                ow = list(si.on_wait) if si is not None and si.on_wait else []
                if limit is not None and len(ow) > limit:
                    keep = ow[-limit:] if limit else []
                    rest = ow[:len(ow) - limit]
                    for j, w in enumerate(rest):
                        ev = mybir.InstEventSemaphore(
                            name=f"{inst.name}_wsplit{j}"
                        )
                        ev.engine = inst.engine
                        ev.sync_info = _br.SyncInfo(on_wait=[w], on_update=[])
                        out.append(ev)
                        n_split += 1
                    inst.sync_info = _br.SyncInfo(
                        on_wait=keep, on_update=list(si.on_update or [])
                    )
                    changed = True
                out.append(inst)
            if changed:
                bb.instructions = out
    return n_split


def _build_program():
    nc = bass.Bass(
        "TRN2", target_bir_lowering=False, debug=False, enable_asserts=False
    )

    ident_d = nc.dram_tensor("ident", [128, 128], BF16,
                             kind="ExternalInput").ap()
    wih_d = nc.dram_tensor("wih", [L, 128, KT * MT * 128], BF16,
                           kind="ExternalInput").ap()
    whh_d = nc.dram_tensor("whh", [L, 128, KT * MT * 128], BF16,
                           kind="ExternalInput").ap()
    bias_d = nc.dram_tensor("bias", [L, 128, MT], F32,
                            kind="ExternalInput").ap()
    x0_d = nc.dram_tensor("x0", [128, T * KT * BL], BF16,
                          kind="ExternalInput").ap()
    h0_d = nc.dram_tensor("h0T", [128, KT * BL], BF16,
                          kind="ExternalInput").ap()
    c0_d = nc.dram_tensor("c0T", [128, KT * BL], F32,
                          kind="ExternalInput").ap()
    wout_d = nc.dram_tensor("wout", [128, KT * VMT * 128], BF16,
                            kind="ExternalInput").ap()
    bout_d = nc.dram_tensor("bout", [128, VMT], F32,
                            kind="ExternalInput").ap()
    lg_d = nc.dram_tensor("logitsT", [VMT, 128, T * BL], F32,
                          kind="ExternalOutput").ap()

    XW = TC * KT * BL     # x-chunk free width  = 1024
    SW = KT * BL          # per-step h width    = 32
    AW = TC * MT * BL     # A-chunk free width  = 4096

    with tile.TileContext(nc) as tc:
        with (
            tc.tile_pool(name="wpool", bufs=8) as wpool,
            tc.tile_pool(name="xpool", bufs=12) as xpool,
            tc.tile_pool(name="apool", bufs=4) as apool,
            tc.tile_pool(name="bpool", bufs=4) as bpool,
            tc.tile_pool(name="const", bufs=1) as cpool,
            tc.tile_pool(name="gpool", bufs=4) as gpool,
            tc.tile_pool(name="spool", bufs=4) as spool,
            tc.tile_pool(name="gps", bufs=6, space="PSUM") as gps,
            tc.tile_pool(name="aps", bufs=2, space="PSUM") as aps,
        ):
            # persistent: initial state + output weights
            h0_t = cpool.tile([128, SW], BF16, tag="h0")
            nc.sync.dma_start(h0_t[:], h0_d)
            c0_t = cpool.tile([128, SW], F32, tag="c0")
            nc.sync.dma_start(c0_t[:], c0_d)
            wout_t = cpool.tile([128, KT * VMT * 128], BF16, tag="wout")
            nc.sync.dma_start(wout_t[:], wout_d)
            bout_t = cpool.tile([128, VMT], F32, tag="bout")
            nc.sync.dma_start(bout_t[:], bout_d)
            bias_all = cpool.tile([128, L * MT], F32, tag="bias_all")
            nc.sync.dma_start(
                bias_all[:], bias_d.rearrange("l p m -> p l m")
            )
            ident_t = cpool.tile([128, 128], BF16, tag="ident")
            nc.sync.dma_start(ident_t[:], ident_d)

            # strict emission-order chaining of the ACT / DVE queues: the
            # Tile scheduler otherwise reorders these streams and parks a
            # psum-blocked op ahead of the critical hn producer, stalling
            # the PE every step (priority inversion).
            from concourse.tile_rust import add_dep_helper as _adh
            chain_prev = {}

            CHAIN_ON = os.environ.get("LSTM_NO_CHAIN", "") != "1"

            def _chain(key, r):
                if not CHAIN_ON:
                    return r
                raw = getattr(r, "ins", r)
                prev = chain_prev.get(key)
                if prev is not None:
                    _adh(raw, prev, sync=False, reason="stream order")
                chain_prev[key] = raw
                return r

            cstate = {}
            for l in range(L):
                cstate[l] = cpool.tile(
                    [128, SW], F32, tag=f"cst{l}", name=f"cst{l}"
                )

            chunk_out = {}

            def emit_prologue(task):
                l, c = task
                st = {}
                wih_t = wpool.tile([128, KT * MT * 128], BF16, tag="w")
                nc.sync.dma_start(wih_t[:], wih_d[l])
                whh_t = wpool.tile([128, KT * MT * 128], BF16, tag="w")
                nc.sync.dma_start(whh_t[:], whh_d[l])
                bias_t = bias_all[:, l * MT:(l + 1) * MT]

                if l == 0:
                    xin = xpool.tile([128, XW], BF16, tag="xchunk")
                    nc.sync.dma_start(xin[:], x0_d[:, c * XW:(c + 1) * XW])
                else:
                    xin = chunk_out[(l - 1, c)]
                xout = xpool.tile([128, XW], BF16, tag="xchunk")
                chunk_out[(l, c)] = xout

                # batched input matmul: A[t,m,b] = (x W_ih^T + b)
                a_t = apool.tile([128, AW], BF16, tag="A")
                xin_k = xin[:].rearrange("p (t k b) -> p k t b", t=TC, k=KT)
                a_mt = a_t[:].rearrange("p (t m b) -> p m t b", t=TC, m=MT)
                for m in range(MT):
                    ps = aps.tile([128, TC * BL], F32, tag="aps")
                    for k in range(KT):
                        nc.tensor.matmul(
                            ps[:],
                            lhsT=wih_t[:, (k * MT + m) * 128:(k * MT + m + 1) * 128],
                            rhs=xin_k[:, k],
                            start=(k == 0),
                            stop=(k == KT - 1),
                        )
                    _chain("act", nc.scalar.activation(
                        a_mt[:, m],
                        ps[:].rearrange("p (t b) -> p t b", t=TC),
                        AF.Identity,
                        bias=bias_t[:, m:m + 1],
                    ))

                st.update(whh=whh_t, xin=xin, xout=xout, A=a_t, l=l, c=c)
                return st

            def emit_step(st, t):
                l = st["l"]
                whh_t = st["whh"]
                xout = st["xout"]

                if t == 0:
                    if st["c"] == 0:
                        rhs = h0_t[:]
                    else:
                        prev = chunk_out[(l, st["c"] - 1)]
                        rhs = prev[:, (TC - 1) * SW:TC * SW]
                else:
                    rhs = xout[:, (t - 1) * SW:t * SW]

                GW = MT * BL          # 128
                QW = GW // 4          # 32 per gate
                g_ps = gps.tile([128, GW], F32, tag="gps")
                # seed PSUM with the precomputed input part via one
                # identity matmul (g = I.T @ A_t), then accumulate the
                # recurrent matmuls on top.  The identity MM must come
                # FIRST: a later start=True would clear has_written for
                # the whole bank and turn accumulation into overwrite.
                nc.tensor.matmul(
                    g_ps[:],
                    lhsT=ident_t[:],
                    rhs=st["A"][:, t * GW:(t + 1) * GW],
                    start=True,
                    stop=False,
                    skip_group_check=True,
                )
                for m in range(MT):
                    for k in range(KT):
                        nc.tensor.matmul(
                            g_ps[:, m * BL:(m + 1) * BL],
                            lhsT=whh_t[:, (k * MT + m) * 128:(k * MT + m + 1) * 128],
                            rhs=rhs[:, k * BL:(k + 1) * BL],
                            start=False,
                            stop=(m == MT - 1 and k == KT - 1),
                            skip_group_check=True,
                        )

                sig = spool.tile([128, 3 * QW], F32, tag="sig")
                _chain("act", nc.scalar.activation(
                    sig[:], g_ps[:, 0:3 * QW], AF.Sigmoid))
                tg = spool.tile([128, QW], F32, tag="tg")
                _chain("act", nc.scalar.activation(
                    tg[:], g_ps[:, 3 * QW:GW], AF.Tanh))

                t1 = spool.tile([128, QW], F32, tag="t1")
                _chain("dve", nc.vector.tensor_mul(t1[:], sig[:, 0:QW], tg[:]))
                t2 = spool.tile([128, QW], F32, tag="t2")
                _chain("dve", nc.vector.tensor_mul(
                    t2[:], sig[:, QW:2 * QW], cstate[l][:]))
                _chain("dve", nc.vector.tensor_add(cstate[l][:], t1[:], t2[:]))
                tcn = spool.tile([128, QW], F32, tag="tcn")
                _chain("act", nc.scalar.activation(
                    tcn[:], cstate[l][:], AF.Tanh))
                _chain("dve", nc.vector.tensor_mul(
                    xout[:, t * SW:(t + 1) * SW], sig[:, 2 * QW:3 * QW], tcn[:]
                ))

            def emit_epilogue(st):
                # output projection for the last layer's chunk
                l, c = st["l"], st["c"]
                if l != L - 1:
                    return
                xo_k = st["xout"][:].rearrange(
                    "p (t k b) -> p k t b", t=TC, k=KT
                )
                for m in range(VMT):
                    ps = aps.tile([128, TC * BL], F32, tag="aps")
                    for k in range(KT):
                        nc.tensor.matmul(
                            ps[:],
                            lhsT=wout_t[:, (k * VMT + m) * 128:(k * VMT + m + 1) * 128],
                            rhs=xo_k[:, k],
                            start=(k == 0),
                            stop=(k == KT - 1),
                        )
                    lg_s = gpool.tile([128, TC * BL], F32, tag="lgout")
                    _chain("act", nc.scalar.activation(
                        lg_s[:], ps[:], AF.Identity, bias=bout_t[:, m:m + 1]
                    ))
                    nc.sync.dma_start(
                        lg_d[m][:, c * TC * BL:(c + 1) * TC * BL], lg_s[:]
                    )

            # c-state init (once, before any task)
            for l in range(L):
                _chain("dve", nc.vector.tensor_copy(cstate[l][:], c0_t[:]))

            for grp in _greedy_groups():
                sts = [emit_prologue(task) for task in grp]
                for t in range(TC):
                    for st in sts:
                        emit_step(st, t)
                for st in sts:
                    emit_epilogue(st)

    _split_excess_waits(nc)
    return nc


# ---------------------------------------------------------------- interface
_CACHE = {}


def _get_program():
    if "nc" not in _CACHE:
        _CACHE["nc"] = _build_program()
    return _CACHE["nc"]


def run(trace=False, **inputs):
    nc = _get_program()
    shared = _prep_shared(inputs)
    in_maps = []
    for ci in range(NCORES):
        m = dict(shared)
        m.update(_prep_core(inputs, ci))
        in_maps.append(m)

    res = run_bass_kernel_spmd(nc, in_maps, list(range(NCORES)), trace=trace)
    out = np.empty((B, T, V), np.float32)
    for ci in range(NCORES):
        out[ci * BL:(ci + 1) * BL] = _unpack_logits(res.results[ci]["logitsT"])
    return out, res


def kernel(**inputs):
    return run(trace=False, **inputs)[0]
